# revision 6
# baseline (speedup 1.0000x reference)
"""Trainium2 Bass kernel for nn_ConvShare: multi-width causal conv + shared projection.

Reference computation (per batch element b):
    xpad = pad(x[b], L -> L+W-1)                       # [L+11, D]
    taps[k]  = xpad[k:k+L, :] @ conv_w[:, :, k].T      # [L, D], k = 0..W-1
    spans[k] = cumsum_k taps                           # [L, D]
    h[k]     = relu(spans[k])
    out[:, k, :] = h[k] @ proj_w.T + proj_b            # [L, W, D]

Sharding: data-parallel over batch B=8 across the 8 NeuronCores (no
communication; conv_w/proj_w replicated per core).

On-chip layout is feature-major ([D, L], contraction dim on SBUF
partitions) for the conv stage; the proj stage uses h as the stationary
matmul operand so its output lands row-major [L, D] and DMAs straight
into the final [L, W, D] layout with 3KB contiguous bursts.

MODE selects the matmul input dtype:
  - "f32r": full fp32 data in the fast fp32 PE mode (1 cycle/row at
    free-dim >= 256). Per-matmul 4-byte LDWEIGHTS (~227ns) is the
    bottleneck (~228us over 1008 matmuls).
  - "bf16": inputs rounded to bf16 (fp32 PSUM accumulate). Enables fast
    weight load and 2x moving-operand streaming.
The conv cumsum (spans) is accumulated in fp32 in both modes.
"""

import os
import sys

import numpy as np

if True:  # make concourse importable regardless of harness cwd
    for _p in ("/opt/trn_rl_repo", "/opt/pypackages"):
        if _p not in sys.path and os.path.isdir(_p):
            sys.path.append(_p)

from contextlib import ExitStack  # noqa: E402

import ml_dtypes  # noqa: E402

import concourse.bacc as bacc  # noqa: E402
import concourse.bass as bass  # noqa: E402
import concourse.mybir as mybir  # noqa: E402
import concourse.tile as tile  # noqa: E402
from concourse import bass_utils  # noqa: E402

B, L, D, W = 8, 512, 768, 12
P = 128          # SBUF partitions
C = D // P       # 6 contraction chunks of 128
LP = L + W - 1   # 523: right-padded sequence length
NB = L // P      # 4 output row blocks for proj

F32 = mybir.dt.float32
RELU = mybir.ActivationFunctionType.Relu

MODE = "f32r"    # "f32r" | "bf16"
CUMSUM = "sbuf"  # "sbuf" | "psum"

# Knobs the test harness may flip before calling kernel():
TRACE = False
LAST_RESULTS = None


def _build_program(mode: str, cumsum: str = "sbuf") -> bass.Bass:
    mdt = mybir.dt.float32r if mode == "f32r" else mybir.dt.bfloat16

    nc = bacc.Bacc(
        "TRN2",
        target_bir_lowering=False,
        debug=False,
        num_devices=B,
    )

    # DRAM I/O. Matmul inputs are pre-chunked host-side to [C, P, n] so each
    # chunk DMA is a clean 2D copy and compute can start on chunk 0 early.
    xT = nc.dram_tensor("xT", [C, P, LP], mdt, kind="ExternalInput").ap()
    cw = nc.dram_tensor("cw", [W, C, P, D], mdt, kind="ExternalInput").ap()
    pw = nc.dram_tensor("pw", [C, P, D], mdt, kind="ExternalInput").ap()
    pb = nc.dram_tensor("pb", [P, D], F32, kind="ExternalInput").ap()
    out = nc.dram_tensor("out", [L, W, D], F32, kind="ExternalOutput").ap()

    with tile.TileContext(nc) as tc, ExitStack() as ctx:
        const_pool = ctx.enter_context(tc.tile_pool(name="const", bufs=1))
        cw_pool = ctx.enter_context(tc.tile_pool(name="cw", bufs=2))
        h_pool = ctx.enter_context(tc.tile_pool(name="h", bufs=2))
        out_pool = ctx.enter_context(tc.tile_pool(name="out", bufs=4))
        if cumsum == "psum":
            psc_pool = ctx.enter_context(tc.tile_pool(name="psc", bufs=1, space="PSUM"))
            psp_pool = ctx.enter_context(tc.tile_pool(name="psp", bufs=2, space="PSUM"))
        else:
            psc_pool = ctx.enter_context(tc.tile_pool(name="psc", bufs=4, space="PSUM"))
            psp_pool = ctx.enter_context(tc.tile_pool(name="psp", bufs=4, space="PSUM"))

        def load_cw(k):
            ts = []
            for c in range(C):
                t = cw_pool.tile([P, D], mdt, tag=f"cw{c}", name=f"cw{c}_{k}")
                nc.sync.dma_start(t[:], cw[k, c, :, :])
                ts.append(t)
            return ts

        # Interleave the startup loads so the first conv matmuls (which need
        # cw[0] chunk c + xT chunk c) can begin as soon as chunk 0 lands.
        cw_cur = []
        xT_t = []
        for c in range(C):
            t = cw_pool.tile([P, D], mdt, tag=f"cw{c}", name=f"cw{c}_0")
            nc.sync.dma_start(t[:], cw[0, c, :, :])
            cw_cur.append(t)
            xt = const_pool.tile([P, LP], mdt, tag=f"xt{c}", name=f"xt{c}")
            nc.sync.dma_start(xt[:], xT[c, :, :])
            xT_t.append(xt)

        pw_t = []
        for c in range(C):
            t = const_pool.tile([P, D], mdt, tag=f"pw{c}", name=f"pw{c}")
            nc.sync.dma_start(t[:], pw[c, :, :])
            pw_t.append(t)
        pb_t = const_pool.tile([P, D], F32)
        nc.sync.dma_start(pb_t[:], pb[:])

        if cumsum == "psum":
            # 6 persistent PSUM banks accumulate the conv cumsum across taps.
            sp_acc = [
                psc_pool.tile([P, L], F32, tag=f"sp{ob}", name=f"sp{ob}")
                for ob in range(C)
            ]
            spans = None
        else:
            spans = const_pool.tile([P, C * L], F32)      # running conv cumsum
            nc.gpsimd.memset(spans[:], 0.0)

        for k in range(W):
            cw_next = load_cw(k + 1) if k + 1 < W else None

            # --- conv tap k: psum[o_blk, l] = sum_d cw^T[d, o] * x^T[d, l+k]
            h_t = [h_pool.tile([P, L], mdt, tag=f"h{c}", name=f"h{c}_{k}") for c in range(C)]
            for ob in range(C):
                if cumsum == "psum":
                    ps = sp_acc[ob]
                    for c in range(C):
                        nc.tensor.matmul(
                            ps[:],
                            lhsT=cw_cur[c][:, ob * P : (ob + 1) * P],
                            rhs=xT_t[c][:, k : k + L],
                            start=(k == 0 and c == 0),
                            stop=(k == W - 1 and c == C - 1),
                            skip_group_check=True,
                        )
                    nc.scalar.activation(h_t[ob][:], ps[:], RELU)
                else:
                    ps = psc_pool.tile([P, L], F32, tag="psc")
                    for c in range(C):
                        nc.tensor.matmul(
                            ps[:],
                            lhsT=cw_cur[c][:, ob * P : (ob + 1) * P],
                            rhs=xT_t[c][:, k : k + L],
                            start=(c == 0),
                            stop=(c == C - 1),
                        )
                    sp = spans[:, ob * L : (ob + 1) * L]
                    nc.vector.tensor_add(sp, sp, ps[:])                    # cumsum
                    nc.scalar.activation(h_t[ob][:], sp, RELU)

            # --- proj tap k: out[l_blk, o2] = sum_d h^T[d, l] * pw^T[d, o2] + b
            for lb in range(NB):
                o_t = out_pool.tile([P, D], F32, tag="out")
                for n0, nn in ((0, 512), (512, 256)):
                    pp = psp_pool.tile([P, 512], F32, tag="psp")
                    for c in range(C):
                        nc.tensor.matmul(
                            pp[:, 0:nn],
                            lhsT=h_t[c][:, lb * P : (lb + 1) * P],
                            rhs=pw_t[c][:, n0 : n0 + nn],
                            start=(c == 0),
                            stop=(c == C - 1),
                        )
                    nc.vector.tensor_add(
                        o_t[:, n0 : n0 + nn], pp[:, 0:nn], pb_t[:, n0 : n0 + nn]
                    )
                nc.sync.dma_start(out[lb * P : (lb + 1) * P, k, :], o_t[:])

            cw_cur = cw_next

    nc.compile()
    return nc


_program_cache: dict = {}


def _get_program(mode: str, cumsum: str = None) -> bass.Bass:
    if cumsum is None:
        cumsum = CUMSUM
    key = (mode, cumsum)
    if key not in _program_cache:
        _program_cache[key] = _build_program(mode, cumsum)
    return _program_cache[key]


def _np_dt(mode: str):
    return np.float32 if mode == "f32r" else ml_dtypes.bfloat16


def _prep_inputs(x, conv_w, proj_w, proj_b, mode: str):
    x = np.asarray(x, dtype=np.float32)
    conv_w = np.asarray(conv_w, dtype=np.float32)
    proj_w = np.asarray(proj_w, dtype=np.float32)
    proj_b = np.asarray(proj_b, dtype=np.float32)
    ndt = _np_dt(mode)

    xT_all = np.zeros((B, D, LP), dtype=np.float32)              # [B, D, L+W-1]
    xT_all[:, :, :L] = x.transpose(0, 2, 1)
    xT_all = np.ascontiguousarray(xT_all.reshape(B, C, P, LP).astype(ndt))
    cwT = np.ascontiguousarray(
        conv_w.transpose(2, 1, 0).reshape(W, C, P, D).astype(ndt)
    )                                                            # [W, C, P, o]
    pwT = np.ascontiguousarray(proj_w.T.reshape(C, P, D).astype(ndt))
    pbb = np.ascontiguousarray(np.broadcast_to(proj_b[None, :], (P, D)))
    return xT_all, cwT, pwT, pbb


def kernel(x, conv_w, proj_w, proj_b):
    global LAST_RESULTS
    nc = _get_program(MODE, CUMSUM)
    xT_all, cwT, pwT, pbb = _prep_inputs(x, conv_w, proj_w, proj_b, MODE)
    in_maps = [
        {"xT": xT_all[b], "cw": cwT, "pw": pwT, "pb": pbb} for b in range(B)
    ]
    res = bass_utils.run_bass_kernel_spmd(
        nc, in_maps, core_ids=list(range(B)), trace=TRACE
    )
    LAST_RESULTS = res
    return np.stack([r["out"] for r in res.results], axis=0)
